# revision 37
# baseline (speedup 1.0000x reference)
"""Gemma sliding-window attention layer on 8 Trainium2 NeuronCores.

Sharding: data-parallel over batch (B=2) x tensor-parallel over heads
(4 groups: 2 q heads + 1 kv head each) = 8 cores. Each core computes a
partial o-proj output [D, S] in bf16; host sums the 4 TP partials per
batch in fp32 and transposes back to [S, D].

Layouts on device (per core):
  q, k: [DH, S] (head-dim on partitions) after rmsnorm+rope, bf16
  v:    [S, DH] (seq on partitions), bf16
  scores^T tiles [k=128, q=256] so no transposes are needed anywhere.

Schedule (per-core): software-pipelined blocks of 512 seq positions:
  proj(b) -> oproj(b-1) -> attn(b), with input DMAs issued one block
  ahead of use and PE warmed up by dummy matmuls during the lead-in.
Softmax denominators accumulate on DVE/GpSimd (bf16 chains) with a
single ones-matmul per q-tile/head; the attention scale 1/16 is folded
into the exp activation so q/k rmsnorm shares one uniform sqrt.
"""

import sys

sys.path.insert(0, "/opt/trn_rl_repo")

from contextlib import ExitStack

import numpy as np
import ml_dtypes

import concourse.bass as bass
import concourse.tile as tile
from concourse import bacc, mybir
from concourse.bass import ds, ts
from concourse.bass_utils import run_bass_kernel_spmd

BF16 = mybir.dt.bfloat16
F32 = mybir.dt.float32
NPBF16 = ml_dtypes.bfloat16
AF = mybir.ActivationFunctionType

H, KVH, DH, SW = 8, 4, 256, 1024
B, S, D = 2, 2048, 2048
EPS = 1e-6
ROPE_BASE = 10000.0
P = 128
SB = 512          # s-block / q-block width
NSB = S // SB     # 4
NDC = D // P      # 16 contraction chunks
EQ = 2 * DH       # per-core q width (2 heads)

ATT_QB = 256      # attention q-tile width
N_WARM = 16       # dummy matmuls to warm the PE p-state during DMA lead-in


def _kchunks(q0, qb):
    """k-chunks (idx, mask_idx|None) needed for q-tile [q0, q0+qb).

    pT tile [k=128 (i), q=qb (j)]: kpos = KC + i, qpos = q0 + j.
    valid iff 0 <= qpos - kpos < SW.
    masks: 0..3 causal (j >= i + 128*o), 4..7 window (j <= i + 128*o - 1);
    q-tiles narrower than 512 use column slices of the same mask set.
    """
    res = []
    for KC in range(max(0, q0 - SW), q0 + qb, P):
        d = KC - q0
        if d >= 0:
            mi = d // P
        elif (d + SW) in (0, 128, 256, 384):
            mi = 4 + (d + SW) // P
        else:
            mi = None
        res.append((KC // P, mi))
    return res


def _build():
    nc = bacc.Bacc("TRN2", target_bir_lowering=False, debug=False)

    xT = nc.dram_tensor("xT", [D, S], BF16, kind="ExternalInput")
    wqT = nc.dram_tensor("wqT", [D, EQ], BF16, kind="ExternalInput")
    wkT = nc.dram_tensor("wkT", [D, DH], BF16, kind="ExternalInput")
    wvT = nc.dram_tensor("wvT", [D, DH], BF16, kind="ExternalInput")
    woT = nc.dram_tensor("woT", [EQ, D], BF16, kind="ExternalInput")
    trig = nc.dram_tensor("trig", [2, P, S], BF16, kind="ExternalInput")  # cos, sin
    masks = nc.dram_tensor("masks", [8, P, SB], BF16, kind="ExternalInput")
    onesd = nc.dram_tensor("onesd", [P, 1], BF16, kind="ExternalInput")
    out = nc.dram_tensor("out", [D, S], BF16, kind="ExternalOutput")

    # eps constant for the rmsnorm sqrt bias; warm-up operand for the PE
    for val in (EPS,):
        t = nc.alloc_sbuf_tensor(f"const-eps-{val}", [P, 1], F32)
        nc.gpsimd.memset(t.ap(), val)
        nc.const_aps.aps[(F32, val)] = t.ap()
    warm = nc.alloc_sbuf_tensor("warm-src", [P, SB], BF16)
    nc.gpsimd.memset(warm.ap(), 1.0)
    nc.all_engine_barrier()

    with tile.TileContext(nc) as tc, ExitStack() as ctx:
        sp = ctx.enter_context(tc.tile_pool(name="sp", bufs=2))    # SBUF
        pp = ctx.enter_context(tc.tile_pool(name="pp", bufs=2, space="PSUM"))

        # ---- persistent SBUF tiles ----
        wq_sb = sp.tile([P, NDC * EQ], BF16, name="wq", tag="wq", bufs=1)
        wk_sb = sp.tile([P, NDC * DH], BF16, name="wk", tag="wk", bufs=1)
        wv_sb = sp.tile([P, NDC * DH], BF16, name="wv", tag="wv", bufs=1)
        wo_sb = sp.tile([P, 4 * D], BF16, name="wo", tag="wo", bufs=1)
        msk_sb = sp.tile([P, 8 * SB], BF16, name="msk", tag="msk", bufs=1)
        ones_sb = sp.tile([P, 1], BF16, name="ones", tag="ones", bufs=1)
        q_sb = [sp.tile([P, S], BF16, name=f"qsb{i}", tag="qsb", bufs=4) for i in range(4)]
        k_sb = [sp.tile([P, S], BF16, name=f"ksb{i}", tag="ksb", bufs=2) for i in range(2)]
        v_sb = [sp.tile([P, DH], BF16, name=f"vsb{i}", tag="vsb", bufs=NDC) for i in range(NDC)]
        ao_sb = [sp.tile([P, S], BF16, name=f"aosb{i}", tag="aosb", bufs=4) for i in range(4)]

        xt_t = {}   # blk -> xt tile
        tg_t = {}   # blk -> trig tile

        xT_r = xT.rearrange("(c p) s -> p c s", p=P)
        trig_r = trig.rearrange("r p s -> p r s")

        def load_xt(blk, split):
            """Prefetch xt/trig for block blk; split xt into `split` DMAs."""
            xt = sp.tile([P, NDC * SB], BF16, name=f"xt{blk}", tag="xt", bufs=2)
            xt_t[blk] = xt
            sbl = ds(blk * SB, SB)
            step = NDC // split
            for g in range(split):
                nc.sync.dma_start(
                    xt[:, ds(g * step * SB, step * SB)].rearrange(
                        "p (c s) -> p c s", c=step),
                    xT_r[:, ds(step * g, step), sbl])
            tgt = sp.tile([P, 2 * SB], BF16, name=f"tg{blk}", tag="tg", bufs=2)
            tg_t[blk] = tgt
            nc.sync.dma_start(tgt[:].rearrange("p (r s) -> p r s", r=2),
                              trig_r[:, :, sbl])

        # ---- block-0 prefetch: interleave wq groups with xt chunks so the
        # first q-projection pass streams without waiting (it consumes
        # (wq g_i, xt chunk i) in lockstep) ----
        xt0 = sp.tile([P, NDC * SB], BF16, name="xt0", tag="xt", bufs=2)
        xt_t[0] = xt0
        for g in range(4):
            nc.sync.dma_start(
                wk_sb[:, ds(g * 4 * DH, 4 * DH)].rearrange("p (c e) -> p c e", c=4),
                wkT.rearrange("(c p) e -> p c e", p=P)[:, ds(4 * g, 4), :])
            nc.sync.dma_start(
                xt0[:, ds(g * 4 * SB, 4 * SB)].rearrange("p (c s) -> p c s", c=4),
                xT_r[:, ds(4 * g, 4), ds(0, SB)])
            if g == 0:
                nc.sync.dma_start(ones_sb[:], onesd[:])
        tgt0 = sp.tile([P, 2 * SB], BF16, name="tg0", tag="tg", bufs=2)
        tg_t[0] = tgt0
        nc.sync.dma_start(tgt0[:].rearrange("p (r s) -> p r s", r=2),
                          trig_r[:, :, ds(0, SB)])
        for g in range(4):
            nc.sync.dma_start(
                wq_sb[:, ds(g * 4 * EQ, 4 * EQ)].rearrange("p (c e) -> p c e", c=4),
                wqT.rearrange("(c p) e -> p c e", p=P)[:, ds(4 * g, 4), :])
        nc.sync.dma_start(msk_sb[:].rearrange("p (m j) -> p m j", m=8),
                          masks.rearrange("m p j -> p m j"))
        nc.sync.dma_start(wv_sb[:].rearrange("p (c e) -> p c e", c=NDC),
                          wvT.rearrange("(c p) e -> p c e", p=P))

        # warm up the PE p-state while the first DMAs land
        for w in range(N_WARM):
            wps = pp.tile([P, SB], F32, name=f"warm{w}", tag="mm", bufs=6)
            nc.tensor.matmul(wps[:], warm.ap()[:, 0:P], warm.ap(),
                             start=True, stop=True)

        def wq_ap(dc, eoff):
            return wq_sb[:, ds(dc * EQ + eoff, P)]

        def wk_ap(dc, eoff):
            return wk_sb[:, ds(dc * DH + eoff, P)]

        out_r = out.rearrange("(g p) s -> p g s", p=P)

        # ---------------- phase emitters ----------------

        def proj(blk, flushes=()):
            """q/k/v projections + rmsnorm + rope for block blk.

            `flushes`: up to two deferred attention-group flushes from the
            previous block, emitted after the first/second ent so their
            normalize chains overlap the remaining projection matmuls."""
            sblice = ds(blk * SB, SB)
            xt = xt_t[blk]
            tcos = tg_t[blk][:, 0:SB]
            tsin = tg_t[blk][:, SB:2 * SB]

            sums = pp.tile([65, SB], F32, name=f"sums{blk}", tag="ao", bufs=2)
            ents = []
            for row, (ent, w_ap, eoff) in enumerate(
                [("k", wk_ap, 0), ("q0", wq_ap, 0), ("q1", wq_ap, DH)]
            ):
                pa = pp.tile([P, SB], F32, name=f"pa_{blk}_{ent}", tag="mm", bufs=6)
                pb = pp.tile([P, SB], F32, name=f"pb_{blk}_{ent}", tag="mm", bufs=6)
                for dc in range(NDC):
                    nc.tensor.matmul(pa[:], w_ap(dc, eoff), xt[:, ds(dc * SB, SB)],
                                     start=(dc == 0), stop=(dc == NDC - 1))
                for dc in range(NDC):
                    nc.tensor.matmul(pb[:], w_ap(dc, eoff + P), xt[:, ds(dc * SB, SB)],
                                     start=(dc == 0), stop=(dc == NDC - 1))
                # bf16 copies feed rope (2x DVE) and free the PSUM quickly
                pab = sp.tile([P, SB], BF16, name=f"pab_{blk}_{ent}", tag="pab", bufs=6)
                pbb = sp.tile([P, SB], BF16, name=f"pbb_{blk}_{ent}", tag="pab", bufs=6)
                nc.vector.tensor_copy(pab[:], pa[:])
                nc.vector.tensor_copy(pbb[:], pb[:])
                sqa = sp.tile([P, SB], BF16, name=f"sqa_{blk}_{ent}", tag="sq", bufs=4)
                sqb = sp.tile([P, SB], BF16, name=f"sqb_{blk}_{ent}", tag="sq", bufs=4)
                nc.vector.tensor_mul(sqa[:], pab[:], pab[:])
                nc.vector.tensor_mul(sqb[:], pbb[:], pbb[:])
                nc.vector.tensor_add(sqa[:], sqa[:], sqb[:])
                roff = 32 * row
                nc.tensor.matmul(sums[roff:roff + 1, :], ones_sb[:], sqa[:],
                                 start=True, stop=True)
                if row < len(flushes):
                    attn_flush(flushes[row])
                ents.append((ent, pab, pbb))

            for sc in range(SB // P):
                pv = pp.tile([P, DH], F32, name=f"pv_{blk}_{sc}", tag="mm", bufs=6)
                for dc in range(NDC):
                    nc.tensor.matmul(pv[:], xt[:, ds(dc * SB + sc * P, P)],
                                     wv_sb[:, ds(dc * DH, DH)],
                                     start=(dc == 0), stop=(dc == NDC - 1))
                nc.scalar.copy(v_sb[blk * (SB // P) + sc][:], pv[:])

            # rinv = 1/sqrt(mean(x^2) + eps), uniform for q rows and k row
            # (the Gemma 1/16 attention scale is folded into the exp below)
            rinv = []
            for row in range(3):
                roff = 32 * row
                rr = sp.tile([1, SB], F32, name=f"rr{blk}_{row}", tag="rr", bufs=3)
                ri = sp.tile([1, SB], BF16, name=f"ri{blk}_{row}", tag="ri", bufs=3)
                nc.scalar.activation(rr[:], sums[roff:roff + 1, :], AF.Sqrt,
                                     bias=EPS, scale=1.0 / DH)
                with nc.allow_low_precision(reason="bf16 rmsnorm scale"):
                    nc.vector.reciprocal(ri[:], rr[:])
                rinv.append(ri)
            # tiny dummy exp: pulls the exp table-set load into the idle
            # mid-proj window instead of stalling the first attention exp
            dume = sp.tile([1, 8], F32, name=f"dume{blk}", tag="dume", bufs=2)
            nc.scalar.activation(dume[:], rinv[2][0:1, 0:8], AF.Exp)

            for row, (ent, pab, pbb) in enumerate(ents):
                rb = sp.tile([P, SB], BF16, name=f"rb_{blk}_{ent}", tag="rb", bufs=3)
                nc.gpsimd.partition_broadcast(rb[:], rinv[row][:])
                if ent == "k":
                    o0, o1 = k_sb[0], k_sb[1]
                else:
                    hh = 0 if ent == "q0" else 1
                    o0, o1 = q_sb[2 * hh], q_sb[2 * hh + 1]
                ta = sp.tile([P, SB], BF16, name=f"ta_{blk}_{ent}", tag="rt", bufs=8)
                tb = sp.tile([P, SB], BF16, name=f"tb_{blk}_{ent}", tag="rt", bufs=8)
                nc.vector.tensor_mul(ta[:], pab[:], tcos)
                nc.vector.tensor_mul(tb[:], pbb[:], tsin)
                nc.vector.tensor_sub(ta[:], ta[:], tb[:])
                nc.vector.tensor_mul(o0[:, sblice], ta[:], rb[:])
                td = sp.tile([P, SB], BF16, name=f"td_{blk}_{ent}", tag="rt", bufs=8)
                te = sp.tile([P, SB], BF16, name=f"te_{blk}_{ent}", tag="rt", bufs=8)
                nc.vector.tensor_mul(td[:], pbb[:], tcos)
                nc.vector.tensor_mul(te[:], pab[:], tsin)
                nc.vector.tensor_add(td[:], td[:], te[:])
                nc.vector.tensor_mul(o1[:, sblice], td[:], rb[:])


        def attn_group(blk, sub, hh, chain_eng, psum_dn=False):
            """Scores+exp+mask+av for one (q-tile, head); returns flush ctx.

            psum_dn=True accumulates the softmax denominator on the PE per
            chunk (used for the final group so its normalize chain starts
            immediately); otherwise a bf16 chain on chain_eng feeds a single
            ones-matmul at flush time."""
            q0 = blk * SB + sub * ATT_QB
            qslice = ds(q0, ATT_QB)
            chunks = _kchunks(q0, ATT_QB)
            nch = len(chunks)
            aop = pp.tile([P, 2 * ATT_QB], F32, name=f"ao_{q0}_{hh}", tag="ao", bufs=2)
            ao0 = aop[:, 0:ATT_QB]
            ao1 = aop[:, ATT_QB:2 * ATT_QB]
            acc = dnp = None
            if psum_dn:
                dnp = pp.tile([1, ATT_QB], F32, name=f"dnp_{q0}_{hh}", tag="mm", bufs=6)
            else:
                acc = sp.tile([P, ATT_QB], BF16, name=f"acc_{q0}_{hh}", tag="acc", bufs=4)
            pts = []
            scps = []
            HB = ATT_QB // 2
            # boundary trims: the leading window-edge chunk (mi==4) is only
            # valid in columns [0,HB); the trailing causal chunk (always
            # mi==1) only in [HB, 2HB)
            trim_first = chunks[0][1] == 4

            def crange(idx):
                if idx == 0 and trim_first:
                    return 0, HB
                if idx == nch - 1:
                    return HB, HB
                return 0, ATT_QB

            def do_scores(idx):
                kc = chunks[idx][0]
                c0, cw = crange(idx)
                scp = pp.tile([P, ATT_QB], F32, name=f"sc_{q0}_{hh}_{kc}",
                              tag="mm", bufs=6)
                nc.tensor.matmul(scp[:, ds(c0, cw)], k_sb[0][:, ts(kc, P)],
                                 q_sb[2 * hh][:, ds(q0 + c0, cw)],
                                 start=True, stop=False)
                nc.tensor.matmul(scp[:, ds(c0, cw)], k_sb[1][:, ts(kc, P)],
                                 q_sb[2 * hh + 1][:, ds(q0 + c0, cw)],
                                 start=False, stop=True)
                scps.append(scp)

            def acc_mms(idx, pt, outs):
                """av (and psum-dn) matmuls for chunk idx. PSUM start/stop
                are bank-granular: only the first matmul into each bank
                carries start=True (marking the whole bank lazily zero) and
                only the last carries stop=True; trimmed columns lazily zero
                on their first touch."""
                first, last = idx == 0, idx == nch - 1
                if trim_first and idx == 1:
                    # the bank was only part-written by the trimmed first
                    # chunk; split so each matmul is uniformly fresh/touched
                    parts = [(0, HB), (HB, HB)]
                else:
                    parts = [crange(idx)]
                for c0, cw in parts:
                    for out, lhsT, st, sp_ in outs:
                        nc.tensor.matmul(out[:, ds(c0, cw)], lhsT,
                                         pt[:, ds(c0, cw)],
                                         start=first and st, stop=last and sp_)

            def do_av(idx):
                kc, mi = chunks[idx]
                c0, cw = crange(idx)
                pt = sp.tile([P, ATT_QB], BF16, name=f"pt_{q0}_{hh}_{kc}",
                             tag="pt", bufs=8)
                nc.scalar.activation(pt[:, ds(c0, cw)], scps[idx][:, ds(c0, cw)],
                                     AF.Exp, scale=1.0 / 16.0)
                if mi is not None:
                    nc.vector.tensor_mul(pt[:, ds(c0, cw)], pt[:, ds(c0, cw)],
                                         msk_sb[:, ds(mi * SB + c0, cw)])
                # ao0 opens the shared aop bank, ao1 closes it
                outs = [(ao0, v_sb[kc][:, 0:P], True, False),
                        (ao1, v_sb[kc][:, P:DH], False, True)]
                if psum_dn:
                    outs.append((dnp, ones_sb[:], True, True))
                acc_mms(idx, pt, outs)
                if not psum_dn:
                    # bf16 denominator chain: full-width over interior chunks;
                    # the trimmed boundary halves are folded in at the end
                    fulls = [i for i in range(nch) if crange(i)[1] == ATT_QB]
                    if idx in fulls:
                        k = fulls.index(idx)
                        if k == 1:
                            chain_eng.tensor_add(acc[:], pts[fulls[0]][:], pt[:])
                        elif k > 1:
                            chain_eng.tensor_add(acc[:], acc[:], pt[:])
                        elif len(fulls) == 1:
                            chain_eng.tensor_copy(acc[:], pt[:])
                    elif idx == nch - 1:
                        if trim_first:
                            chain_eng.tensor_add(acc[:, ds(0, HB)],
                                                 acc[:, ds(0, HB)],
                                                 pts[0][:, ds(0, HB)])
                        chain_eng.tensor_add(acc[:, ds(HB, HB)],
                                             acc[:, ds(HB, HB)], pt[:, ds(HB, HB)])
                pts.append(pt)

            la = min(5, nch - 1)
            for idx in range(nch):
                do_scores(idx)
                if idx >= la:
                    do_av(idx - la)
            for idx in range(nch - la, nch):
                do_av(idx)
            return (blk, sub, hh, qslice, ao0, ao1, acc, dnp)

        def attn_flush(g):
            """Denominator matmul + normalize for a finished group."""
            blk, sub, hh, qslice, ao0, ao1, acc, dnp = g
            if dnp is None:
                dnp = pp.tile([1, ATT_QB], F32, name=f"dn_{blk}_{sub}_{hh}",
                              tag="mm", bufs=6)
                nc.tensor.matmul(dnp[:], ones_sb[:], acc[:], start=True, stop=True)
            dr = sp.tile([1, ATT_QB], F32, name=f"dr_{blk}_{sub}_{hh}", tag="dr", bufs=2)
            nc.vector.reciprocal(dr[:], dnp[:])
            db = sp.tile([P, ATT_QB], F32, name=f"db_{blk}_{sub}_{hh}", tag="db", bufs=4)
            nc.gpsimd.partition_broadcast(db[:], dr[:])
            nc.vector.tensor_mul(ao_sb[2 * hh][:, qslice], ao0, db[:])
            nc.vector.tensor_mul(ao_sb[2 * hh + 1][:, qslice], ao1, db[:])

        def oproj(blk, halves=False):
            """o-proj for block blk. halves=True: per-sub N=256 matmuls
            (used for the final block's first half to shorten the tail)."""
            sblice = ds(blk * SB, SB)
            for g4 in range(4):
                ob4 = sp.tile([P, 4 * SB], BF16, name=f"ob_{blk}_{g4}", tag="ob", bufs=3)
                for j in range(4):
                    dmc = 4 * g4 + j
                    op = pp.tile([P, SB], F32, name=f"op_{blk}_{dmc}", tag="mm", bufs=6)
                    for ec in range(4):
                        nc.tensor.matmul(op[:], wo_sb[:, ds(ec * D + dmc * P, P)],
                                         ao_sb[ec][:, sblice],
                                         start=(ec == 0), stop=(ec == 3))
                    if j % 2 == 0:
                        nc.scalar.copy(ob4[:, ds(j * SB, SB)], op[:])
                    else:
                        nc.vector.tensor_copy(ob4[:, ds(j * SB, SB)], op[:])
                nc.sync.dma_start(
                    out_r[:, ds(4 * g4, 4), sblice],
                    ob4[:].rearrange("p (g s) -> p g s", g=4))

        def oproj_sub(blk, sub, late_hi=False):
            """o-proj for one q-sub-tile (N=256) of block blk, fully
            contracted per tile so PSUM tiles rotate; copy + store.

            late_hi=True: emit the ec0/ec1 matmuls of all four tiles in a
            group before any ec2/ec3, buying time for the last attention
            group (heads 2,3) to finish normalizing."""
            qsl = ds(blk * SB + sub * ATT_QB, ATT_QB)
            for g4 in range(4):
                ob = sp.tile([P, 4 * ATT_QB], BF16,
                             name=f"obs_{blk}_{sub}_{g4}", tag="ob", bufs=3)
                ops = []
                for j in range(4):
                    dmc = 4 * g4 + j
                    op = pp.tile([P, ATT_QB], F32,
                                 name=f"ops_{blk}_{sub}_{dmc}", tag="mm", bufs=6)
                    ops.append((dmc, op))
                ec_order = ([(j, ec) for ec in (0, 1) for j in range(4)]
                            + [(j, ec) for ec in (2, 3) for j in range(4)]
                            if late_hi and g4 == 0 else
                            [(j, ec) for j in range(4) for ec in range(4)])
                tail = late_hi and g4 == 3
                for j, ec in ec_order:
                    dmc, op = ops[j]
                    nc.tensor.matmul(op[:], wo_sb[:, ds(ec * D + dmc * P, P)],
                                     ao_sb[ec][:, qsl],
                                     start=(ec == 0), stop=(ec == 3))
                    if tail and ec == 3:
                        # inline copy + half-DMAs so the very last store chain
                        # is as short as possible
                        if j % 2 == 0:
                            nc.scalar.copy(ob[:, ds(j * ATT_QB, ATT_QB)], op[:])
                        else:
                            nc.vector.tensor_copy(ob[:, ds(j * ATT_QB, ATT_QB)], op[:])
                        if j % 2 == 1:
                            nc.sync.dma_start(
                                out_r[:, ds(4 * g4 + j - 1, 2), qsl],
                                ob[:, ds((j - 1) * ATT_QB, 2 * ATT_QB)].rearrange(
                                    "p (g s) -> p g s", g=2))
                if not tail:
                    for j in range(4):
                        dmc, op = ops[j]
                        if j % 2 == 0:
                            nc.scalar.copy(ob[:, ds(j * ATT_QB, ATT_QB)], op[:])
                        else:
                            nc.vector.tensor_copy(ob[:, ds(j * ATT_QB, ATT_QB)], op[:])
                    nc.sync.dma_start(
                        out_r[:, ds(4 * g4, 4), qsl],
                        ob[:].rearrange("p (g s) -> p g s", g=4))

        # ---------------- main schedule ----------------
        pending = []
        ch_dve = nc.vector
        ch_pool = nc.gpsimd
        for blk in range(NSB):
            # prefetch next block's inputs ahead of this block's out-DMAs
            if blk + 1 < NSB:
                load_xt(blk + 1, 2)
            if blk == 0:
                nc.sync.dma_start(wo_sb[:].rearrange("p (c e) -> p c e", c=4),
                                  woT.rearrange("(c p) e -> p c e", p=P))

            # the previous block's last two attention groups flush inside
            # proj: their normalize chains hide under the projection matmuls
            proj(blk, flushes=tuple(pending))
            pending = []

            if blk > 0:
                oproj(blk - 1)

            if blk < NSB - 1:
                for sub in range(SB // ATT_QB):
                    for hh in range(2):
                        g = attn_group(blk, sub, hh, ch_dve)
                        if len(pending) >= 1:
                            attn_flush(pending.pop(0))
                        pending.append(g)
            else:
                # final block: all chains on DVE (GpSimd adds are too slow
                # for the short flush windows here), per-sub o-proj
                # interleaved to shorten the tail
                g00 = attn_group(blk, 0, 0, ch_dve)
                g01 = attn_group(blk, 0, 1, ch_dve)
                attn_flush(g00)
                g10 = attn_group(blk, 1, 0, ch_dve)
                attn_flush(g01)
                g11 = attn_group(blk, 1, 1, ch_dve, psum_dn=True)
                attn_flush(g10)
                attn_flush(g11)
                oproj_sub(blk, 0)
                oproj_sub(blk, 1, late_hi=True)

    nc.compile()
    return nc


_NC = None
LAST_RESULT = None


def _get_nc():
    global _NC
    if _NC is None:
        _NC = _build()
    return _NC


def _host_tables(q_norm_w, k_norm_w):
    qw, kw = np.asarray(q_norm_w, np.float64), np.asarray(k_norm_w, np.float64)
    # device shares one cos/sin table across q/k and both rotary halves;
    # requires uniform (1 + w) factors (true for Gemma-zero-init norm weights)
    assert np.allclose(qw, qw[0]) and np.allclose(kw, kw[0]) and np.allclose(qw[0], kw[0]), \
        "non-uniform q/k norm weights need the 8-row trig layout"
    c = 1.0 + qw[0]
    inv_freq = 1.0 / (ROPE_BASE ** (np.arange(0, DH, 2, dtype=np.float64) / DH))
    freqs = np.outer(np.arange(S, dtype=np.float64), inv_freq)   # [S, DH/2]
    cos = (np.cos(freqs) * c).T.astype(np.float32)               # [DH/2, S]
    sin = (np.sin(freqs) * c).T.astype(np.float32)
    trig = np.stack([cos, sin]).astype(NPBF16)                   # [2, 128, S]

    i = np.arange(P)[:, None]
    j = np.arange(SB)[None, :]
    mrows = [(j >= i + P * o) for o in range(4)] + [(j <= i + P * o - 1) for o in range(4)]
    masks = np.stack(mrows).astype(NPBF16)
    ones = np.ones((P, 1), NPBF16)
    return trig, masks, ones


def kernel(hidden_states, Wq, Wk, Wv, Wo, q_norm_w, k_norm_w):
    global LAST_RESULT
    nc = _get_nc()
    trig, masks, ones = _host_tables(np.asarray(q_norm_w), np.asarray(k_norm_w))

    xTs = [np.ascontiguousarray(np.asarray(hidden_states)[b].T).astype(NPBF16)
           for b in range(B)]
    in_maps = []
    for core in range(8):
        b, g = core // 4, core % 4
        in_maps.append({
            "xT": xTs[b],
            "wqT": np.ascontiguousarray(np.asarray(Wq)[g * EQ:(g + 1) * EQ, :].T).astype(NPBF16),
            "wkT": np.ascontiguousarray(np.asarray(Wk)[g * DH:(g + 1) * DH, :].T).astype(NPBF16),
            "wvT": np.ascontiguousarray(np.asarray(Wv)[g * DH:(g + 1) * DH, :].T).astype(NPBF16),
            "woT": np.ascontiguousarray(np.asarray(Wo)[:, g * EQ:(g + 1) * EQ].T).astype(NPBF16),
            "trig": trig,
            "masks": masks,
            "onesd": ones,
        })

    LAST_RESULT = run_bass_kernel_spmd(nc, in_maps, list(range(8)))
    res = LAST_RESULT.results
    outs = []
    for b in range(B):
        acc = np.zeros((D, S), np.float32)
        for g in range(4):
            acc += res[4 * b + g]["out"].astype(np.float32)
        outs.append(acc.T)
    return np.stack(outs).astype(np.float32)
